# revision 1
# baseline (speedup 1.0000x reference)
"""Grouped Conv2D (32 groups of 8->8 ch, 3x3, SAME) on 8 trn2 NeuronCores.

Strategy:
  - Data-parallel over batch: 32 images / 8 cores = 4 images per core.
  - Grouped conv as implicit GEMM: for each of the 9 taps, a 128x128
    block-diagonal stationary (16 groups x [8ic x 8oc] blocks) multiplies a
    tap-shifted view of the zero-haloed input image, accumulating in PSUM.
  - bf16 inputs (host-cast) / bf16 weights, fp32 PSUM accumulate.
  - Per-core: 4 images x 2 channel-chunks x 7 row-strips x 9 taps matmuls.
"""

import sys

import numpy as np

if "/opt/trn_rl_repo" not in sys.path:
    sys.path.insert(0, "/opt/trn_rl_repo")

import ml_dtypes

B, C, H, W = 32, 256, 56, 56
KK = 3
GROUPS = 32
CPG = 8  # in- and out-channels per group
N_CORES = 8
BPC = B // N_CORES  # images per core
HP, WP = H + 2, W + 2  # padded image
NCHUNK = 2  # 256 channels = 2 x 128 partitions
GPC = 16  # groups per chunk
STRIP = 8  # output rows per PSUM strip (8*56=448 <= 512 fp32/bank)
NSTRIP = H // STRIP
# strips grouped into multi-bank psum tiles: (start_strip, n_strips)
PSUM_GROUPS = [(0, 4), (4, 3)]


def _pack_weights(w: np.ndarray) -> np.ndarray:
    """[256, 8, 3, 3] fp32 -> [128 pc, 2 chunk, 9 tap, 128 po] bf16 block-diag.

    lhsT[pc=8*gl+ic, po=8*gl+oc_local] = w[128*ck + 8*gl + oc_local, ic, th, tw]
    """
    wr = w.reshape(NCHUNK, GPC, CPG, CPG, KK, KK)  # ck, gl, o, ic, th, tw
    wpk = np.zeros((GPC, CPG, NCHUNK, KK * KK, GPC, CPG), dtype=np.float32)
    for gl in range(GPC):
        # [ck, o, ic, th, tw] -> [ic, ck, (th tw), o]
        blk = wr[:, gl].transpose(2, 0, 3, 4, 1).reshape(CPG, NCHUNK, KK * KK, CPG)
        wpk[gl, :, :, :, gl, :] = blk
    return wpk.reshape(128, NCHUNK, KK * KK, 128).astype(ml_dtypes.bfloat16)


def _build_bass():
    import concourse.tile as tile
    from concourse import bacc, mybir

    nc = bacc.Bacc()
    xs = nc.dram_tensor(
        "xs", [BPC, C, HP, WP], mybir.dt.bfloat16, kind="ExternalInput"
    )
    wpk = nc.dram_tensor(
        "wpk", [128, NCHUNK, KK * KK, 128], mybir.dt.bfloat16, kind="ExternalInput"
    )
    out = nc.dram_tensor("out", [BPC, C, H, W], mybir.dt.float32, kind="ExternalOutput")

    with tile.TileContext(nc) as tc:
        with (
            tc.tile_pool(name="singles", bufs=1) as singles,
            tc.tile_pool(name="xpad_pool", bufs=3) as xpad_pool,
            tc.tile_pool(name="ot_pool", bufs=4) as ot_pool,
            tc.tile_pool(name="psum_pool", bufs=2, space="PSUM") as psum_pool,
        ):
            # HW-DGE lane order: w, in0, out_img0..2, out_img3_ck0/ck1A/ck1B (8)
            # SW-DGE lane order: in1..in7 (7) -- zero lane reuse, so every DMA
            # carries at most one sync wait (walrus hard limit). xpad bufs=3
            # throttles later inputs via buffer-reuse waits (their only wait),
            # avoiding HBM contention with the critical first tiles.
            w_sb = singles.tile([128, NCHUNK, KK * KK, 128], mybir.dt.bfloat16)
            nc.sync.dma_start(out=w_sb[:], in_=wpk[:])

            # PE warm-up: data-independent matmuls on the weight tile so the
            # HAM clock-gate releases (1.2 -> 2.4 GHz) before real work lands.
            wu = psum_pool.tile([128, 4, 512], mybir.dt.float32, name="ps")
            for i in range(4):
                nc.tensor.matmul(
                    wu[:, i, :],
                    lhsT=w_sb[:, 0, 0, :],
                    rhs=w_sb[:, 0, 0:4, :],
                    start=True,
                    stop=True,
                )

            def do_chunk(xpad, ck, sink):
                """Matmul both psum groups of one (image, chunk); sink(group
                index, psum tile, ns) consumes each accumulated group."""
                for gi, (s0, ns) in enumerate(PSUM_GROUPS):
                    ps = psum_pool.tile([128, 4, 512], mybir.dt.float32, name="ps")
                    for t in range(KK * KK):
                        th, tw = divmod(t, KK)
                        for si in range(ns):
                            s = s0 + si
                            nc.tensor.matmul(
                                ps[:, si, : STRIP * W],
                                lhsT=w_sb[:, ck, t, :],
                                rhs=xpad[
                                    :,
                                    s * STRIP + th : s * STRIP + th + STRIP,
                                    tw : tw + W,
                                ],
                                start=(t == 0),
                                stop=(t == KK * KK - 1),
                            )
                    sink(gi, ps, s0, ns)

            # group-A matmuls of (0,0) only need padded rows 0..33; splitting
            # the first input at that boundary halves time-to-first-matmul
            # (subtile deps let group A start on the first half alone).
            SPLIT = PSUM_GROUPS[0][1] * STRIP + 2  # 34

            def load_xpad(b, ck):
                xpad = xpad_pool.tile([128, HP, WP], mybir.dt.bfloat16, name="xpad")
                if (b, ck) == (0, 0):
                    nc.sync.dma_start(
                        out=xpad[:, :SPLIT, :],
                        in_=xs[b, ck * 128 : (ck + 1) * 128, :SPLIT, :],
                    )
                    nc.gpsimd.dma_start(
                        out=xpad[:, SPLIT:, :],
                        in_=xs[b, ck * 128 : (ck + 1) * 128, SPLIT:, :],
                    )
                else:
                    nc.gpsimd.dma_start(
                        out=xpad[:], in_=xs[b, ck * 128 : (ck + 1) * 128]
                    )
                return xpad

            xpads = {(b, ck): load_xpad(b, ck) for b in range(BPC) for ck in range(NCHUNK)}

            for b in range(BPC - 1):
                ot = ot_pool.tile([128, NCHUNK, H, W], mybir.dt.float32, name="oti", bufs=3)
                for ck in range(NCHUNK):
                    def sink(gi, ps, s0, ns, _ot=ot, _ck=ck):
                        nc.scalar.copy(
                            out=_ot[:, _ck, s0 * STRIP : (s0 + ns) * STRIP, :],
                            in_=ps[:, :ns, : STRIP * W],
                        )
                    do_chunk(xpads[(b, ck)], ck, sink)
                dst = out[b].rearrange("(ck c) h w -> c ck h w", ck=NCHUNK)
                nc.sync.dma_start(out=dst, in_=ot[:])

            # last image: finer-grained drain so the tail exposes only the
            # final 24-row group (copied + DMA'd on the spare HW lane).
            b = BPC - 1
            ot3 = ot_pool.tile([128, H, W], mybir.dt.float32, name="ot3", bufs=1)
            def sink3a(gi, ps, s0, ns):
                nc.scalar.copy(
                    out=ot3[:, s0 * STRIP : (s0 + ns) * STRIP, :],
                    in_=ps[:, :ns, : STRIP * W],
                )
            do_chunk(xpads[(b, 0)], 0, sink3a)
            nc.sync.dma_start(out=out[b, 0:128], in_=ot3[:])

            def sink3b(gi, ps, s0, ns):
                otg = ot_pool.tile([128, 4 * STRIP, W], mybir.dt.float32, name="otg", bufs=2)
                nc.scalar.copy(
                    out=otg[:, : ns * STRIP, :], in_=ps[:, :ns, : STRIP * W]
                )
                nc.sync.dma_start(
                    out=out[b, 128:256, s0 * STRIP : (s0 + ns) * STRIP, :],
                    in_=otg[:, : ns * STRIP, :],
                )
            do_chunk(xpads[(b, 1)], 1, sink3b)
    nc.finalize()
    return nc


_CACHE = {}


def kernel(x, w, trace=False):
    from concourse.bass_utils import run_bass_kernel_spmd

    x = np.asarray(x)
    w = np.ascontiguousarray(np.asarray(w), dtype=np.float32)

    if "nc" not in _CACHE:
        _CACHE["nc"] = _build_bass()
    nc = _CACHE["nc"]

    xbf = np.zeros((B, C, HP, WP), dtype=ml_dtypes.bfloat16)
    xbf[:, :, 1 : H + 1, 1 : W + 1] = x.astype(ml_dtypes.bfloat16)
    wpk = _pack_weights(w)
    in_maps = [
        {"xs": np.ascontiguousarray(xbf[i * BPC : (i + 1) * BPC]), "wpk": wpk}
        for i in range(N_CORES)
    ]
    res = run_bass_kernel_spmd(
        nc, in_maps, core_ids=list(range(N_CORES)), trace=trace
    )
    out = np.concatenate([res.results[i]["out"] for i in range(N_CORES)], axis=0)
    if trace:
        kernel.last_result = res
    return out



# revision 3
# speedup vs baseline: 2.2294x; 2.2294x over previous
"""Grouped Conv2D (32 groups of 8->8 ch, 3x3, SAME) on 8 trn2 NeuronCores.

Strategy (v2 - dense-contraction implicit GEMM):
  - Data-parallel over batch: 32 images / 8 cores = 4 images (2 pairs) per core.
  - Per group g, pack 14 consecutive output ROWS into PE output partitions:
      po = (oc, pr)  : 8 out-ch x 14 rows = 112 outputs
      pc = (ic, rho) : 8 in-ch x 16 input rows = 128 -> contraction fully dense
    The 3 column-taps (tw) are 3 accumulating matmuls with col-shifted rhs
    views; the 3 row-taps live inside the (rho - pr) Toeplitz structure of the
    stationary. 192 matmuls of [128x128 @ 128x448] per core vs 504 for the
    16-group block-diagonal scheme (2.6x fewer PE columns).
  - Host pre-packs input into [pc, pair, g, (img,rgrp), col] bf16 so every DMA
    is a dense [128 x contiguous] rectangle; output returned bf16 in matmul
    layout and unscrambled + upcast on host.
"""

import sys

import numpy as np

if "/opt/trn_rl_repo" not in sys.path:
    sys.path.insert(0, "/opt/trn_rl_repo")

import ml_dtypes

B, C, H, W = 32, 256, 56, 56
KK = 3
GROUPS = 32
CPG = 8  # in- and out-channels per group
N_CORES = 8
BPC = B // N_CORES  # images per core
NPAIR = 2  # image pairs per core
PR = 14  # output rows per partition-block
RG = 4  # row groups (4 x 14 = 56 rows)
RHO = 16  # input rows per row group (14 + 2 halo)
PO = CPG * PR  # 112 output partitions
FREE = 2 * RG * W  # 448 matmul columns: (img, rgrp, col)
WP = W + 2  # padded cols


def _pack_weights(w: np.ndarray) -> np.ndarray:
    """[256, 8, 3, 3] fp32 -> [128 pc, 32 g, 3 tw, 112 po] bf16 block-Toeplitz.

    wpk[(ic,rho), g, tw, (oc,pr)] = w[8g+oc, ic, rho-pr, tw] for rho-pr in 0..2
    """
    wr = w.reshape(GROUPS, CPG, CPG, KK, KK)  # g, oc, ic, th, tw
    wl = np.zeros((CPG, RHO, GROUPS, KK, CPG, PR), np.float32)
    for th in range(KK):
        src = wr[:, :, :, th, :].transpose(2, 0, 3, 1)  # ic, g, tw, oc
        for pr in range(PR):
            wl[:, pr + th, :, :, :, pr] = src
    return wl.reshape(128, GROUPS, KK, PO).astype(ml_dtypes.bfloat16)


def _pack_inputs(x: np.ndarray) -> np.ndarray:
    """[32, 256, 56, 56] fp32 -> [8 core, 128 pc, 2 pair, 32 g, 8 m, 58 c] bf16.

    xin[core, (ic,rho), pair, g, (img,rgrp), c] = xpad[b, 8g+ic, 14*rgrp+rho, c]
    with b = 4*core + 2*pair + img and xpad zero-padded by 1 in rows/cols.
    """
    xpad = np.zeros((B, C, H + 2, WP), dtype=ml_dtypes.bfloat16)
    xpad[:, :, 1 : H + 1, 1 : W + 1] = x.astype(ml_dtypes.bfloat16)
    rows = PR * np.arange(RG)[:, None] + np.arange(RHO)[None, :]  # [rgrp, rho]
    xg = xpad.reshape(N_CORES, NPAIR, 2, GROUPS, CPG, H + 2, WP)
    xr = xg[:, :, :, :, :, rows, :]  # core, pair, img, g, ic, rgrp, rho, c
    return np.ascontiguousarray(
        xr.transpose(0, 4, 6, 1, 3, 2, 5, 7).reshape(
            N_CORES, 128, NPAIR, GROUPS, 2 * RG, WP
        )
    )


def _unpack_output(outs) -> np.ndarray:
    """per-core [112, 2, 32, 448] bf16 -> [32, 256, 56, 56] fp32."""
    o = np.stack([np.asarray(t) for t in outs])  # core, (oc,pr), pair, g, (img,rgrp,c)
    o = o.reshape(N_CORES, CPG, PR, NPAIR, GROUPS, 2, RG, W)
    o = o.transpose(0, 3, 5, 4, 1, 6, 2, 7)  # core, pair, img, g, oc, rgrp, pr, c
    return np.ascontiguousarray(o.reshape(B, C, H, W)).astype(np.float32)


def _build_bass():
    import concourse.tile as tile
    from concourse import bacc, mybir

    nc = bacc.Bacc()
    xin = nc.dram_tensor(
        "xin", [128, NPAIR, GROUPS, 2 * RG, WP], mybir.dt.bfloat16,
        kind="ExternalInput",
    )
    wpk = nc.dram_tensor(
        "wpk", [128, GROUPS, KK, PO], mybir.dt.bfloat16, kind="ExternalInput"
    )
    out = nc.dram_tensor(
        "out", [PO, NPAIR, GROUPS, FREE], mybir.dt.bfloat16, kind="ExternalOutput"
    )

    # group-range chunking: weights and inputs interleaved on the sync HWDGE
    # ring so each arrives just ahead of the matmuls that consume it.
    GCHUNKS = [(0, 2), (2, 6), (6, 11), (11, 16), (16, 24), (24, 32)]

    with tile.TileContext(nc) as tc:
        with (
            tc.tile_pool(name="singles", bufs=1) as singles,
            tc.tile_pool(name="psum_pool", bufs=8, space="PSUM") as psum_pool,
        ):
            X = singles.tile([128, NPAIR, GROUPS, 2 * RG, WP], mybir.dt.bfloat16)
            Wt = singles.tile([128, GROUPS, KK, PO], mybir.dt.bfloat16)
            O = singles.tile([PO, NPAIR, GROUPS, FREE], mybir.dt.bfloat16)

            for a, b in GCHUNKS:
                nc.sync.dma_start(out=Wt[:, a:b], in_=wpk[:, a:b])
                nc.sync.dma_start(out=X[:, 0, a:b], in_=xin[:, 0, a:b])
            for a in range(0, GROUPS, 8):
                nc.sync.dma_start(out=X[:, 1, a : a + 8], in_=xin[:, 1, a : a + 8])

            # PE warm-up on the first weight chunk so the HAM clock gate
            # (1.2 -> 2.4 GHz) releases before real matmuls land.
            wu = psum_pool.tile([128, FREE], mybir.dt.float32, name="ps")
            for _ in range(4):
                nc.tensor.matmul(
                    wu[:PO, : KK * PO],
                    lhsT=Wt[:, 0, 0, :],
                    rhs=Wt[:, 0],
                    start=True,
                    stop=True,
                )

            copy_engines = [nc.scalar, nc.vector]
            for pair in range(NPAIR):
                for blk in range(GROUPS // 8):
                    # one engine per 8-group block: its FIFO order makes the
                    # trailing output DMA depend on a single engine's copies.
                    eng = copy_engines[blk % 2]
                    for g in range(8 * blk, 8 * blk + 8):
                        ps = psum_pool.tile([128, FREE], mybir.dt.float32, name="ps")
                        for tw in range(KK):
                            nc.tensor.matmul(
                                ps[:PO, :],
                                lhsT=Wt[:, g, tw, :],
                                rhs=X[:, pair, g, :, tw : tw + W],
                                start=(tw == 0),
                                stop=(tw == KK - 1),
                            )
                        if eng is nc.scalar:
                            eng.copy(out=O[:, pair, g], in_=ps[:PO, :])
                        else:
                            eng.tensor_copy(out=O[:, pair, g], in_=ps[:PO, :])
                    dma_eng = nc.scalar if eng is nc.scalar else nc.gpsimd
                    dma_eng.dma_start(
                        out=out[:, pair, 8 * blk : 8 * blk + 8],
                        in_=O[:, pair, 8 * blk : 8 * blk + 8],
                    )
    nc.finalize()
    return nc


_CACHE = {}


def kernel(x, w, trace=False):
    from concourse.bass_utils import run_bass_kernel_spmd

    x = np.ascontiguousarray(np.asarray(x), dtype=np.float32)
    w = np.ascontiguousarray(np.asarray(w), dtype=np.float32)

    if "nc" not in _CACHE:
        _CACHE["nc"] = _build_bass()
    nc = _CACHE["nc"]

    xin = _pack_inputs(x)
    wp = _pack_weights(w)
    in_maps = [{"xin": xin[i], "wpk": wp} for i in range(N_CORES)]
    res = run_bass_kernel_spmd(
        nc, in_maps, core_ids=list(range(N_CORES)), trace=trace
    )
    outp = _unpack_output([res.results[i]["out"] for i in range(N_CORES)])
    if trace:
        kernel.last_result = res
    return outp


# revision 7
# speedup vs baseline: 2.4051x; 1.0788x over previous
"""Grouped Conv2D (32 groups of 8->8 ch, 3x3, SAME) on 8 trn2 NeuronCores.

Strategy (v4 - dense-contraction implicit GEMM, group-sharded):
  - Shard over channel GROUPS: each core owns 4 of the 32 groups for all 32
    images (expert-style parallelism). Same FLOPs/bytes for activations as
    batch sharding, but per-core weight traffic drops 8x (0.34 MB).
  - Per group g, pack 14 consecutive output ROWS into PE output partitions:
      po = (oc, pr)  : 8 out-ch x 14 rows = 112 outputs
      pc = (ic, rho) : 8 in-ch x 16 input rows = 128 -> contraction fully dense
    The 3 column-taps (tw) are 3 accumulating matmuls with col-shifted rhs
    views; the 3 row-taps live inside the (rho - pr) Toeplitz structure of the
    stationary. 192 matmuls of [128x128 @ 128x448] per core vs 504 for the
    16-group block-diagonal scheme (2.6x fewer PE columns).
  - Host pre-packs input into [pc, pair, g, (img,rgrp), col] bf16 so every DMA
    is a dense [128 x contiguous] rectangle; output returned bf16 in matmul
    layout and unscrambled + upcast on host.
  - All DMAs ride one sync HWDGE ring: inputs first, outputs FIFO behind them,
    so input feed has strict HBM priority and completions stay on the cheap
    HWDGE path.
"""

import sys

import numpy as np

if "/opt/trn_rl_repo" not in sys.path:
    sys.path.insert(0, "/opt/trn_rl_repo")

import ml_dtypes

B, C, H, W = 32, 256, 56, 56
KK = 3
GROUPS = 32
CPG = 8  # in- and out-channels per group
N_CORES = 8
GPC = GROUPS // N_CORES  # groups per core
NPAIR = B // 2  # image pairs per core (all 32 images, 16 pairs)
PR = 14  # output rows per partition-block
RG = 4  # row groups (4 x 14 = 56 rows)
RHO = 16  # input rows per row group (14 + 2 halo)
PO = CPG * PR  # 112 output partitions
FREE = 2 * RG * W  # 448 matmul columns: (img, rgrp, col)
WP = W + 2  # padded cols


def _pack_weights(w: np.ndarray) -> np.ndarray:
    """[256, 8, 3, 3] fp32 -> [128 pc, 32 g, 3 tw, 112 po] bf16 block-Toeplitz.

    wpk[(ic,rho), g, tw, (oc,pr)] = w[8g+oc, ic, rho-pr, tw] for rho-pr in 0..2
    """
    wr = w.reshape(GROUPS, CPG, CPG, KK, KK)  # g, oc, ic, th, tw
    wl = np.zeros((CPG, RHO, GROUPS, KK, CPG, PR), np.float32)
    for th in range(KK):
        src = wr[:, :, :, th, :].transpose(2, 0, 3, 1)  # ic, g, tw, oc
        for pr in range(PR):
            wl[:, pr + th, :, :, :, pr] = src
    return wl.reshape(128, GROUPS, KK, PO).astype(ml_dtypes.bfloat16)


def _pack_inputs(x: np.ndarray) -> np.ndarray:
    """[32, 256, 56, 56] fp32 -> [8 core, 128 pc, 16 pair, 4 g, 8 m, 58 c] bf16.

    xin[core, (ic,rho), pair, g, (img,rgrp), c] = xpad[b, ch, 14*rgrp+rho, c]
    with b = 2*pair + img, ch = 8*(4*core + g) + ic, xpad zero-padded by 1.
    """
    xpad = np.zeros((B, C, H + 2, WP), dtype=ml_dtypes.bfloat16)
    xpad[:, :, 1 : H + 1, 1 : W + 1] = x.astype(ml_dtypes.bfloat16)
    rows = PR * np.arange(RG)[:, None] + np.arange(RHO)[None, :]  # [rgrp, rho]
    xg = xpad.reshape(NPAIR, 2, N_CORES, GPC, CPG, H + 2, WP)
    xr = xg[:, :, :, :, :, rows, :]  # pair, img, core, g, ic, rgrp, rho, c
    return np.ascontiguousarray(
        xr.transpose(2, 4, 6, 0, 3, 1, 5, 7).reshape(
            N_CORES, 128, NPAIR, GPC, 2 * RG, WP
        )
    )


def _unpack_output(outs) -> np.ndarray:
    """per-core [112, 16, 4, 448] bf16 -> [32, 256, 56, 56] fp32."""
    o = np.stack([np.asarray(t) for t in outs])
    o = o.reshape(N_CORES, CPG, PR, NPAIR, GPC, 2, RG, W)
    o = o.transpose(3, 5, 0, 4, 1, 6, 2, 7)  # pair,img,core,g,oc,rgrp,pr,c
    return np.ascontiguousarray(o.reshape(B, C, H, W)).astype(np.float32)


def _build_bass():
    import concourse.tile as tile
    from concourse import bacc, mybir

    nc = bacc.Bacc()
    xin = nc.dram_tensor(
        "xin", [128, NPAIR, GPC, 2 * RG, WP], mybir.dt.bfloat16,
        kind="ExternalInput",
    )
    wpk = nc.dram_tensor(
        "wpk", [128, GPC, KK, PO], mybir.dt.bfloat16, kind="ExternalInput"
    )
    out = nc.dram_tensor(
        "out", [PO, NPAIR, GPC, FREE], mybir.dt.bfloat16, kind="ExternalOutput"
    )

    # input chunking along image pairs; sized so the stream stays ahead of the
    # PE (consumption ~200 GB/s < one-ring DMA ~358 GB/s).
    PCHUNKS = [(1, 3), (3, 6), (6, 10), (10, 16)]

    with tile.TileContext(nc) as tc:
        with (
            tc.tile_pool(name="singles", bufs=1) as singles,
            tc.tile_pool(name="psum_pool", bufs=8, space="PSUM") as psum_pool,
        ):
            X = singles.tile([128, NPAIR, GPC, 2 * RG, WP], mybir.dt.bfloat16)
            Wt = singles.tile([128, GPC, KK, PO], mybir.dt.bfloat16)
            O = singles.tile([PO, NPAIR, GPC, FREE], mybir.dt.bfloat16)

            # PE warm-up on a memset tile so the HAM clock gate (1.2 ->
            # 2.4 GHz) starts releasing before the first weights even land.
            wz = singles.tile([128, 128], mybir.dt.bfloat16)
            nc.vector.memset(wz[:], 0)
            wu = psum_pool.tile([128, FREE], mybir.dt.float32, name="ps")
            for _ in range(12):
                nc.tensor.matmul(
                    wu[:, :128], lhsT=wz[:], rhs=wz[:], start=True, stop=True
                )

            nc.sync.dma_start(out=Wt[:], in_=wpk[:])
            nc.sync.dma_start(out=X[:, 0, 0:1], in_=xin[:, 0, 0:1])
            nc.sync.dma_start(out=X[:, 0, 1:], in_=xin[:, 0, 1:])
            for a, b in PCHUNKS:
                nc.sync.dma_start(out=X[:, a:b], in_=xin[:, a:b])

            copy_engines = [nc.scalar, nc.vector]
            for pair in range(NPAIR):
                # one engine per pair: its FIFO order lets the trailing output
                # DMA depend on a single engine's copies (one sync wait).
                eng = copy_engines[pair % 2]
                for g in range(GPC):
                    ps = psum_pool.tile([128, FREE], mybir.dt.float32, name="ps")
                    for tw in range(KK):
                        nc.tensor.matmul(
                            ps[:PO, :],
                            lhsT=Wt[:, g, tw, :],
                            rhs=X[:, pair, g, :, tw : tw + W],
                            start=(tw == 0),
                            stop=(tw == KK - 1),
                        )
                    if eng is nc.scalar:
                        eng.copy(out=O[:, pair, g], in_=ps[:PO, :])
                    else:
                        eng.tensor_copy(out=O[:, pair, g], in_=ps[:PO, :])
                nc.sync.dma_start(
                    out=out[:, pair], in_=O[:, pair]
                )
    nc.finalize()
    return nc


_CACHE = {}


def kernel(x, w, trace=False):
    from concourse.bass_utils import run_bass_kernel_spmd

    x = np.ascontiguousarray(np.asarray(x), dtype=np.float32)
    w = np.ascontiguousarray(np.asarray(w), dtype=np.float32)

    if "nc" not in _CACHE:
        _CACHE["nc"] = _build_bass()
    nc = _CACHE["nc"]

    xin = _pack_inputs(x)
    wp = _pack_weights(w)
    in_maps = [
        {"xin": xin[i], "wpk": np.ascontiguousarray(wp[:, GPC * i : GPC * (i + 1)])}
        for i in range(N_CORES)
    ]
    res = run_bass_kernel_spmd(
        nc, in_maps, core_ids=list(range(N_CORES)), trace=trace
    )
    outp = _unpack_output([res.results[i]["out"] for i in range(N_CORES)])
    if trace:
        kernel.last_result = res
    return outp
